# revision 7
# baseline (speedup 1.0000x reference)
"""Graphormer multi-head attention on 8 Trainium2 cores.

Sharding: 2 cores per batch element (B=4), each core handling 8 of 16 heads
(tensor-parallel within the batch). Per core, a software pipeline over head
PAIRS (hc = 0..3, heads 2hc/2hc+1):

  cycle hc: QK matmuls for the pair, row-tiled so the even head (PE rows
  0-63) and odd head (rows 64-127) run concurrently; EXP on ACT; p =
  exp(scores)*expbT on DVE; PE filler = PV of pair hc-1 + q/k projections
  of pair hc+1 (+ first output-projection half in the last cycle).

This overlaps the projection GEMMs with the EXP chain (the serial ACT
bottleneck) instead of running them as separate phases, and doubles QK
throughput via PE row-group concurrency. PV keeps the ones-column trick
(row 64 of the accumulator = softmax denominator). Outputs ship as two
bf16 partials (feature halves), summed on host with the pair core's.
"""
import sys

sys.path.insert(0, '/opt/trn_rl_repo')

from collections import deque

import ml_dtypes
import numpy as np

import concourse.bass as bass
import concourse.mybir as mybir
import concourse.tile as tile
from concourse import bacc
from concourse.bass_utils import run_bass_kernel_spmd

DT = mybir.dt

B, T, S, E, H = 4, 1024, 1024, 1024, 16
D = E // H          # 64
HL = 8              # heads per core
F = HL * D          # 512 local features
N_CORES = 8

MMDT = DT.bfloat16
NP_MMDT = ml_dtypes.bfloat16


def _build_program():
    nc = bacc.Bacc()

    xqT = nc.dram_tensor("xqT", [E, T], MMDT, kind="ExternalInput")
    xkT = nc.dram_tensor("xkT", [E, S], MMDT, kind="ExternalInput")
    xvT = nc.dram_tensor("xvT", [E, S], MMDT, kind="ExternalInput")
    wqT = nc.dram_tensor("wqT", [E, F], MMDT, kind="ExternalInput")
    wkT = nc.dram_tensor("wkT", [E, F], MMDT, kind="ExternalInput")
    wvT = nc.dram_tensor("wvT", [E, F], MMDT, kind="ExternalInput")
    woT = nc.dram_tensor("woT", [F, E], MMDT, kind="ExternalInput")
    bq = nc.dram_tensor("bq", [128, 4], DT.float32, kind="ExternalInput")
    bk = nc.dram_tensor("bk", [128, 4], DT.float32, kind="ExternalInput")
    bv = nc.dram_tensor("bv", [1, F], DT.float32, kind="ExternalInput")
    expbT = nc.dram_tensor("expbT", [S, T], MMDT, kind="ExternalInput")
    yT = nc.dram_tensor("yT", [E, T], MMDT, kind="ExternalOutput")
    yTa = nc.dram_tensor("yTa", [E, T], MMDT, kind="ExternalOutput")

    with tile.TileContext(nc) as tc:
        with tc.tile_pool(name="pp", bufs=1) as pp, \
             tc.tile_pool(name="pTp", bufs=22) as pTp, \
             tc.tile_pool(name="wkp", bufs=2) as wk, \
             tc.tile_pool(name="nrm", bufs=1) as nrm, \
             tc.tile_pool(name="psqk", bufs=2, space="PSUM") as psqk, \
             tc.tile_pool(name="pspv", bufs=2, space="PSUM") as pspv, \
             tc.tile_pool(name="pm", bufs=1, space="PSUM") as pm:

            # ---- ACT exp-table warm-up while DMAs run ----
            warm_in = pp.tile([1, 8], DT.float32, tag="warm_in")
            warm_out = pp.tile([1, 8], DT.float32, tag="warm_out")
            nc.vector.memset(warm_in[:], 0.0)
            nc.scalar.activation(warm_out[:], warm_in[:],
                                 mybir.ActivationFunctionType.Exp)

            # ---- input DMAs, critical-path order, chunked ----
            wq_sb = pp.tile([128, 8, F], MMDT, tag="wq")
            wqT_v = wqT[:].rearrange("(c p) f -> p c f", p=128)
            xq_sb = pp.tile([128, 8, T], MMDT, tag="xq")
            xqT_v = xqT[:].rearrange("(c p) t -> p c t", p=128)
            for ec in range(8):
                nc.sync.dma_start(wq_sb[:, ec, :], wqT_v[:, ec, :])
                nc.sync.dma_start(xq_sb[:, ec, :], xqT_v[:, ec, :])
            bq_sb = pp.tile([128, 4], DT.float32, tag="bq")
            nc.sync.dma_start(bq_sb[:], bq[:])

            wk_sb = pp.tile([128, 8, F], MMDT, tag="wk")
            wkT_v = wkT[:].rearrange("(c p) f -> p c f", p=128)
            xk_sb = pp.tile([128, 8, S], MMDT, tag="xk")
            xkT_v = xkT[:].rearrange("(c p) t -> p c t", p=128)
            for ec in range(8):
                nc.sync.dma_start(wk_sb[:, ec, :], wkT_v[:, ec, :])
                nc.sync.dma_start(xk_sb[:, ec, :], xkT_v[:, ec, :])
            bk_sb = pp.tile([128, 4], DT.float32, tag="bk")
            nc.sync.dma_start(bk_sb[:], bk[:])

            expb_sb = pp.tile([128, 8, T], MMDT, tag="expb")
            expbT_v = expbT[:].rearrange("(c p) t -> p c t", p=128)
            for ec in range(8):
                nc.sync.dma_start(expb_sb[:, ec, :], expbT_v[:, ec, :])

            wv_sb = pp.tile([128, 8, F], MMDT, tag="wv")
            wvT_v = wvT[:].rearrange("(c p) f -> p c f", p=128)
            xv_sb = pp.tile([128, 8, S], MMDT, tag="xv")
            xvT_v = xvT[:].rearrange("(c p) t -> p c t", p=128)
            for ec in range(8):
                nc.sync.dma_start(wv_sb[:, ec, :], wvT_v[:, ec, :])
                nc.sync.dma_start(xv_sb[:, ec, :], xvT_v[:, ec, :])
            bv_row = pp.tile([1, F], DT.float32, tag="bvrow")
            nc.sync.dma_start(bv_row[:], bv[:])
            wo_sb = pp.tile([128, 4, E], MMDT, tag="wo")
            nc.sync.dma_start(wo_sb[:], woT[:].rearrange("(c p) e -> p c e", p=128))

            # ---- persistent SBUF state ----
            qT_sb = pp.tile([128, 4, T], MMDT, tag="qT")
            kT_sb = pp.tile([128, 4, S], MMDT, tag="kT")
            v_sb = pp.tile([128, 8, HL * 65], MMDT, tag="v")
            oT_sb = pp.tile([128, 4, T], MMDT, tag="oT")
            bv_bc = pp.tile([128, F], DT.float32, tag="bvbc")
            nc.gpsimd.partition_broadcast(bv_bc[:], bv_row[:])

            # ---- emission helpers ----
            fillers = deque()

            def drain(n):
                for _ in range(n):
                    if fillers:
                        fillers.popleft()()

            def proj_closures(kind, fc):
                x_sb, w_sb, b_sb, dst = {
                    'q': (xq_sb, wq_sb, bq_sb, qT_sb),
                    'k': (xk_sb, wk_sb, bk_sb, kT_sb),
                }[kind]
                box = {}

                def mms(th):
                    def go():
                        if th == 0:
                            box['acc'] = pm.tile([128, T], DT.float32, tag="mm", name="acc")
                        acc = box['acc']
                        for ec in range(8):
                            nc.tensor.matmul(
                                acc[:, th * 512:(th + 1) * 512],
                                w_sb[:, ec, fc * 128:(fc + 1) * 128],
                                x_sb[:, ec, th * 512:(th + 1) * 512],
                                start=(ec == 0), stop=(ec == 7),
                            )
                        nc.vector.tensor_scalar_add(
                            dst[:, fc, th * 512:(th + 1) * 512],
                            acc[:, th * 512:(th + 1) * 512],
                            b_sb[:, fc:fc + 1],
                        )
                    return go
                return [mms(0), mms(1)]

            def v_closure(sc):
                def go():
                    acc = pm.tile([128, T], DT.float32, tag="mm")
                    for ec in range(8):
                        nc.tensor.matmul(
                            acc[:, 0:F],
                            xv_sb[:, ec, sc * 128:(sc + 1) * 128],
                            wv_sb[:, ec, :],
                            start=(ec == 0), stop=(ec == 7),
                        )
                    vv = v_sb[:, sc, :].rearrange("p (h c) -> p h c", c=65)
                    nc.vector.tensor_add(
                        vv[:, :, 0:64],
                        acc[:, 0:F].rearrange("p (h d) -> p h d", d=64),
                        bv_bc[:].rearrange("p (h d) -> p h d", d=64),
                    )
                    nc.vector.memset(vv[:, :, 64:65], 1.0)
                return go

            chunks = {}   # (head, sc) -> pT tile [128, T]

            def qk_step(hc, sc):
                tA = psqk.tile([128, T], DT.float32, tag="qk")
                tB = psqk.tile([128, T], DT.float32, tag="qk")
                for th in range(2):
                    nc.tensor.matmul(
                        tA[:, th * 512:(th + 1) * 512],
                        kT_sb[0:64, hc, sc * 128:(sc + 1) * 128],
                        qT_sb[0:64, hc, th * 512:(th + 1) * 512],
                        start=True, stop=True,
                    )
                    nc.tensor.matmul(
                        tB[:, th * 512:(th + 1) * 512],
                        kT_sb[64:128, hc, sc * 128:(sc + 1) * 128],
                        qT_sb[64:128, hc, th * 512:(th + 1) * 512],
                        start=True, stop=True,
                    )
                for h, t in ((2 * hc, tA), (2 * hc + 1, tB)):
                    et = wk.tile([128, T], MMDT, tag="et")
                    nc.scalar.activation(et[:], t[:],
                                         mybir.ActivationFunctionType.Exp)
                    p = pTp.tile([128, T], MMDT, tag="pT")
                    nc.vector.tensor_mul(p[:], et[:], expb_sb[:, sc, :])
                    chunks[(h, sc)] = p

            def pv_closures(h):
                box = {}

                def chunk(sc):
                    def go():
                        if sc == 0:
                            box['a0'] = pspv.tile([65, 512], DT.float32, tag="pv", name="pva")
                            box['a1'] = pspv.tile([65, 512], DT.float32, tag="pv", name="pvb")
                        p = chunks.pop((h, sc))
                        for th, acc in ((0, box['a0']), (1, box['a1'])):
                            nc.tensor.matmul(
                                acc[:],
                                v_sb[:, sc, h * 65:(h + 1) * 65],
                                p[:, th * 512:(th + 1) * 512],
                                start=(sc == 0), stop=(sc == 7),
                            )
                    return go

                def finish():
                    hc, po = h // 2, 64 * (h % 2)
                    osb = wk.tile([65, T], DT.float32, tag="osb")
                    nc.vector.tensor_copy(osb[:, 0:512], box['a0'][:])
                    nc.vector.tensor_copy(osb[:, 512:1024], box['a1'][:])
                    lrow = nrm.tile([1, T], DT.float32, tag="lrow")
                    nc.vector.tensor_copy(lrow[:], osb[64:65, :])
                    rlb = nrm.tile([64, T], DT.float32, tag="rlb")
                    nc.gpsimd.partition_broadcast(rlb[:], lrow[:])
                    rli = nrm.tile([64, T], DT.float32, tag="rli")
                    nc.vector.reciprocal_approx_fast(out=rli[:], in_=rlb[:])
                    nc.vector.tensor_mul(oT_sb[po:po + 64, hc, :],
                                         osb[0:64, :], rli[:])

                return [chunk(sc) for sc in range(8)] + [finish]

            def y_closure(half, ec8, out_t):
                fcs = (0, 1) if half == 0 else (2, 3)

                def go():
                    yps = pm.tile([128, T], DT.float32, tag="mm")
                    for th in range(2):
                        for fc in fcs:
                            nc.tensor.matmul(
                                yps[:, th * 512:(th + 1) * 512],
                                wo_sb[:, fc, ec8 * 128:(ec8 + 1) * 128],
                                oT_sb[:, fc, th * 512:(th + 1) * 512],
                                start=(fc == fcs[0]), stop=(fc == fcs[1]),
                            )
                    ysb = wk.tile([128, T], MMDT, tag="ysb")
                    nc.vector.tensor_copy(ysb[:], yps[:])
                    nc.sync.dma_start(out_t[ec8 * 128:(ec8 + 1) * 128, :], ysb[:])
                return go

            # ---- prologue: q/k fc0 inline ----
            for cl in proj_closures('q', 0) + proj_closures('k', 0):
                cl()

            # ---- pipelined pair cycles ----
            for hc in range(4):
                if hc == 0:
                    new = [v_closure(sc) for sc in range(8)]
                    new += proj_closures('q', 1) + proj_closures('k', 1)
                else:
                    pvA = pv_closures(2 * (hc - 1))
                    pvB = pv_closures(2 * (hc - 1) + 1)
                    new = pvA + pvB
                    if hc < 3:
                        new += proj_closures('q', hc + 1) + proj_closures('k', hc + 1)
                    else:
                        new += [y_closure(0, ec8, yTa) for ec8 in range(8)]
                fillers.extend(new)
                for sc in range(8):
                    qk_step(hc, sc)
                    rem = len(fillers)
                    drain(-(-rem // (8 - sc)))
                drain(len(fillers))

            # ---- epilogue: PV pair 3, then second output half ----
            for cl in pv_closures(6) + pv_closures(7):
                cl()
            for ec8 in range(8):
                y_closure(1, ec8, yT)()

    nc.compile()
    return nc


_NC_CACHE = []


def kernel(query, key_, value, edge_bias, attn_mask, key_padding_mask,
           Wq, bq, Wk, bk, Wv, bv, Wo, bo):
    if not _NC_CACHE:
        _NC_CACHE.append(_build_program())
    nc = _NC_CACHE[0]

    scale = np.float32(D ** -0.5)
    q32, k32, v32 = (np.asarray(a, np.float32) for a in (query, key_, value))
    WqT = (np.asarray(Wq, np.float32).T * scale).astype(NP_MMDT)
    WkT = np.asarray(Wk, np.float32).T.astype(NP_MMDT)
    WvT = np.asarray(Wv, np.float32).T.astype(NP_MMDT)
    WoT = np.asarray(Wo, np.float32).T
    bq_s = (np.asarray(bq, np.float32) * scale)
    kpm_add = np.where(np.asarray(key_padding_mask), np.float32(-1e30),
                       np.float32(0.0))  # [B, S]
    mask32 = np.asarray(attn_mask, np.float32)

    in_maps = []
    for c in range(N_CORES):
        b, g = divmod(c, 2)
        cols = slice(g * F, (g + 1) * F)
        bias_sb = (mask32 + np.asarray(edge_bias[b], np.float32)
                   + kpm_add[b][None, :])  # [T, S]
        in_maps.append({
            "xqT": np.ascontiguousarray(q32[b].T).astype(NP_MMDT),
            "xkT": np.ascontiguousarray(k32[b].T).astype(NP_MMDT),
            "xvT": np.ascontiguousarray(v32[b].T).astype(NP_MMDT),
            "wqT": np.ascontiguousarray(WqT[:, cols]),
            "wkT": np.ascontiguousarray(WkT[:, cols]),
            "wvT": np.ascontiguousarray(WvT[:, cols]),
            "woT": np.ascontiguousarray(WoT[cols, :]).astype(NP_MMDT),
            "bq": np.ascontiguousarray(bq_s[cols].reshape(4, 128).T),
            "bk": np.ascontiguousarray(np.asarray(bk, np.float32)[cols]
                                       .reshape(4, 128).T),
            "bv": np.asarray(bv, np.float32)[cols].reshape(1, F),
            "expbT": np.exp(bias_sb.T).astype(NP_MMDT),
        })

    res = run_bass_kernel_spmd(nc, in_maps, list(range(N_CORES)))

    out = np.empty((B, T, E), np.float32)
    bo32 = np.asarray(bo, np.float32)
    for b in range(B):
        r0, r1 = res.results[2 * b], res.results[2 * b + 1]
        acc = (r0["yT"].astype(np.float32) + r0["yTa"].astype(np.float32)
               + r1["yT"].astype(np.float32) + r1["yTa"].astype(np.float32))
        out[b] = acc.T + bo32[None, :]
    return out


# revision 8
# speedup vs baseline: 1.0218x; 1.0218x over previous
"""Graphormer multi-head attention on 8 Trainium2 cores.

Sharding: 2 cores per batch element (B=4), each core handling 8 of 16 heads
(tensor-parallel within the batch). Per core, a software pipeline over head
PAIRS (hc = 0..3, heads 2hc/2hc+1):

  cycle hc: QK matmuls for the pair, row-tiled so the even head (PE rows
  0-63) and odd head (rows 64-127) run concurrently; EXP on ACT; p =
  exp(scores)*expbT on DVE; PE filler = PV of pair hc-1 + q/k projections
  of pair hc+1 (+ first output-projection half in the last cycle).

This overlaps the projection GEMMs with the EXP chain (the serial ACT
bottleneck) instead of running them as separate phases, and doubles QK
throughput via PE row-group concurrency. PV keeps the ones-column trick
(row 64 of the accumulator = softmax denominator). Outputs ship as two
bf16 partials (feature halves), summed on host with the pair core's.
"""
import sys

sys.path.insert(0, '/opt/trn_rl_repo')

from collections import deque

import ml_dtypes
import numpy as np

import concourse.bass as bass
import concourse.mybir as mybir
import concourse.tile as tile
from concourse import bacc
from concourse.bass_utils import run_bass_kernel_spmd

DT = mybir.dt

B, T, S, E, H = 4, 1024, 1024, 1024, 16
D = E // H          # 64
HL = 8              # heads per core
F = HL * D          # 512 local features
N_CORES = 8

MMDT = DT.bfloat16
NP_MMDT = ml_dtypes.bfloat16


def _build_program():
    nc = bacc.Bacc()

    xqT = nc.dram_tensor("xqT", [E, T], MMDT, kind="ExternalInput")
    xkT = nc.dram_tensor("xkT", [E, S], MMDT, kind="ExternalInput")
    xvT = nc.dram_tensor("xvT", [E, S], MMDT, kind="ExternalInput")
    wqT = nc.dram_tensor("wqT", [E, F], MMDT, kind="ExternalInput")
    wkT = nc.dram_tensor("wkT", [E, F], MMDT, kind="ExternalInput")
    wvT = nc.dram_tensor("wvT", [E, F], MMDT, kind="ExternalInput")
    woT = nc.dram_tensor("woT", [F, E], MMDT, kind="ExternalInput")
    bq = nc.dram_tensor("bq", [128, 4], DT.float32, kind="ExternalInput")
    bk = nc.dram_tensor("bk", [128, 4], DT.float32, kind="ExternalInput")
    bv = nc.dram_tensor("bv", [1, F], DT.float32, kind="ExternalInput")
    expbT = nc.dram_tensor("expbT", [S, T], MMDT, kind="ExternalInput")
    yT = nc.dram_tensor("yT", [E, T], MMDT, kind="ExternalOutput")
    yTa = nc.dram_tensor("yTa", [E, T], MMDT, kind="ExternalOutput")

    with tile.TileContext(nc) as tc:
        with tc.tile_pool(name="pp", bufs=1) as pp, \
             tc.tile_pool(name="pTp", bufs=22) as pTp, \
             tc.tile_pool(name="wkp", bufs=2) as wk, \
             tc.tile_pool(name="nrm", bufs=1) as nrm, \
             tc.tile_pool(name="psqk", bufs=2, space="PSUM") as psqk, \
             tc.tile_pool(name="pspv", bufs=2, space="PSUM") as pspv, \
             tc.tile_pool(name="pm", bufs=1, space="PSUM") as pm:

            # ---- ACT exp-table warm-up while DMAs run ----
            warm_in = pp.tile([1, 8], DT.float32, tag="warm_in")
            warm_out = pp.tile([1, 8], DT.float32, tag="warm_out")
            nc.vector.memset(warm_in[:], 0.0)
            nc.scalar.activation(warm_out[:], warm_in[:],
                                 mybir.ActivationFunctionType.Exp)

            # ---- input DMAs, critical-path order, chunked ----
            wq_sb = pp.tile([128, 8, F], MMDT, tag="wq")
            wqT_v = wqT[:].rearrange("(c p) f -> p c f", p=128)
            xq_sb = pp.tile([128, 8, T], MMDT, tag="xq")
            xqT_v = xqT[:].rearrange("(c p) t -> p c t", p=128)
            for ec in range(8):
                nc.sync.dma_start(wq_sb[:, ec, :], wqT_v[:, ec, :])
                nc.sync.dma_start(xq_sb[:, ec, :], xqT_v[:, ec, :])
            bq_sb = pp.tile([128, 4], DT.float32, tag="bq")
            nc.sync.dma_start(bq_sb[:], bq[:])

            wk_sb = pp.tile([128, 8, F], MMDT, tag="wk")
            wkT_v = wkT[:].rearrange("(c p) f -> p c f", p=128)
            xk_sb = pp.tile([128, 8, S], MMDT, tag="xk")
            xkT_v = xkT[:].rearrange("(c p) t -> p c t", p=128)
            for ec in range(8):
                nc.sync.dma_start(wk_sb[:, ec, :], wkT_v[:, ec, :])
                nc.sync.dma_start(xk_sb[:, ec, :], xkT_v[:, ec, :])
            bk_sb = pp.tile([128, 4], DT.float32, tag="bk")
            nc.sync.dma_start(bk_sb[:], bk[:])

            expb_sb = pp.tile([128, 8, T], MMDT, tag="expb")
            expbT_v = expbT[:].rearrange("(c p) t -> p c t", p=128)
            for ec in range(8):
                nc.sync.dma_start(expb_sb[:, ec, :], expbT_v[:, ec, :])

            wv_sb = pp.tile([128, 8, F], MMDT, tag="wv")
            wvT_v = wvT[:].rearrange("(c p) f -> p c f", p=128)
            xv_sb = pp.tile([128, 8, S], MMDT, tag="xv")
            xvT_v = xvT[:].rearrange("(c p) t -> p c t", p=128)
            for ec in range(8):
                nc.sync.dma_start(wv_sb[:, ec, :], wvT_v[:, ec, :])
                nc.sync.dma_start(xv_sb[:, ec, :], xvT_v[:, ec, :])
            bv_row = pp.tile([1, F], DT.float32, tag="bvrow")
            nc.sync.dma_start(bv_row[:], bv[:])
            wo_sb = pp.tile([128, 4, E], MMDT, tag="wo")
            nc.sync.dma_start(wo_sb[:], woT[:].rearrange("(c p) e -> p c e", p=128))

            # ---- persistent SBUF state ----
            qT_sb = pp.tile([128, 4, T], MMDT, tag="qT")
            kT_sb = pp.tile([128, 4, S], MMDT, tag="kT")
            v_sb = pp.tile([128, 8, HL * 65], MMDT, tag="v")
            oT_sb = pp.tile([128, 4, T], MMDT, tag="oT")
            bv_bc = pp.tile([128, F], DT.float32, tag="bvbc")
            nc.gpsimd.partition_broadcast(bv_bc[:], bv_row[:])

            # ---- emission helpers ----
            fillers = deque()

            def drain(n):
                for _ in range(n):
                    if fillers:
                        fillers.popleft()()

            def proj_closures(kind, fc):
                x_sb, w_sb, b_sb, dst = {
                    'q': (xq_sb, wq_sb, bq_sb, qT_sb),
                    'k': (xk_sb, wk_sb, bk_sb, kT_sb),
                }[kind]

                def go():
                    acc = pm.tile([128, T], DT.float32, tag="mm", name="acc")
                    for th in range(2):
                        for ec in range(8):
                            nc.tensor.matmul(
                                acc[:, th * 512:(th + 1) * 512],
                                w_sb[:, ec, fc * 128:(fc + 1) * 128],
                                x_sb[:, ec, th * 512:(th + 1) * 512],
                                start=(ec == 0), stop=(ec == 7),
                            )
                    if kind == 'q':
                        nc.scalar.add(dst[:, fc, :], acc[:], b_sb[:, fc:fc + 1])
                    else:
                        nc.vector.tensor_scalar_add(dst[:, fc, :], acc[:],
                                                    b_sb[:, fc:fc + 1])
                return [go]

            def v_closure(sc):
                def go():
                    acc = pm.tile([128, T], DT.float32, tag="mm")
                    for ec in range(8):
                        nc.tensor.matmul(
                            acc[:, 0:F],
                            xv_sb[:, ec, sc * 128:(sc + 1) * 128],
                            wv_sb[:, ec, :],
                            start=(ec == 0), stop=(ec == 7),
                        )
                    vv = v_sb[:, sc, :].rearrange("p (h c) -> p h c", c=65)
                    nc.vector.tensor_add(
                        vv[:, :, 0:64],
                        acc[:, 0:F].rearrange("p (h d) -> p h d", d=64),
                        bv_bc[:].rearrange("p (h d) -> p h d", d=64),
                    )
                    nc.vector.memset(vv[:, :, 64:65], 1.0)
                return go

            chunks = {}   # (head, sc) -> pT tile [128, T]

            def qk_step(hc, sc):
                tA = psqk.tile([128, T], DT.float32, tag="qk")
                tB = psqk.tile([128, T], DT.float32, tag="qk")
                for th in range(2):
                    nc.tensor.matmul(
                        tA[:, th * 512:(th + 1) * 512],
                        kT_sb[0:64, hc, sc * 128:(sc + 1) * 128],
                        qT_sb[0:64, hc, th * 512:(th + 1) * 512],
                        start=True, stop=True,
                    )
                    nc.tensor.matmul(
                        tB[:, th * 512:(th + 1) * 512],
                        kT_sb[64:128, hc, sc * 128:(sc + 1) * 128],
                        qT_sb[64:128, hc, th * 512:(th + 1) * 512],
                        start=True, stop=True,
                    )
                for h, t in ((2 * hc, tA), (2 * hc + 1, tB)):
                    et = wk.tile([128, T], MMDT, tag="et")
                    nc.scalar.activation(et[:], t[:],
                                         mybir.ActivationFunctionType.Exp)
                    p = pTp.tile([128, T], MMDT, tag="pT")
                    nc.vector.tensor_mul(p[:], et[:], expb_sb[:, sc, :])
                    chunks[(h, sc)] = p

            def pv_closures(h):
                box = {}

                def chunk(sc):
                    def go():
                        if sc == 0:
                            box['a0'] = pspv.tile([65, 512], DT.float32, tag="pv", name="pva")
                            box['a1'] = pspv.tile([65, 512], DT.float32, tag="pv", name="pvb")
                        p = chunks.pop((h, sc))
                        for th, acc in ((0, box['a0']), (1, box['a1'])):
                            nc.tensor.matmul(
                                acc[:],
                                v_sb[:, sc, h * 65:(h + 1) * 65],
                                p[:, th * 512:(th + 1) * 512],
                                start=(sc == 0), stop=(sc == 7),
                            )
                    return go

                def finish():
                    hc, po = h // 2, 64 * (h % 2)
                    osb = wk.tile([65, T], DT.float32, tag="osb")
                    nc.vector.tensor_copy(osb[:, 0:512], box['a0'][:])
                    nc.vector.tensor_copy(osb[:, 512:1024], box['a1'][:])
                    lrow = nrm.tile([1, T], DT.float32, tag="lrow")
                    nc.vector.tensor_copy(lrow[:], osb[64:65, :])
                    rlb = nrm.tile([64, T], DT.float32, tag="rlb")
                    nc.gpsimd.partition_broadcast(rlb[:], lrow[:])
                    rli = nrm.tile([64, T], DT.float32, tag="rli")
                    nc.vector.reciprocal_approx_fast(out=rli[:], in_=rlb[:])
                    nc.vector.tensor_mul(oT_sb[po:po + 64, hc, :],
                                         osb[0:64, :], rli[:])

                return [chunk(sc) for sc in range(8)] + [finish]

            def y_closure(half, ec8, out_t):
                fcs = (0, 1) if half == 0 else (2, 3)

                def go():
                    yps = pm.tile([128, T], DT.float32, tag="mm")
                    for th in range(2):
                        for fc in fcs:
                            nc.tensor.matmul(
                                yps[:, th * 512:(th + 1) * 512],
                                wo_sb[:, fc, ec8 * 128:(ec8 + 1) * 128],
                                oT_sb[:, fc, th * 512:(th + 1) * 512],
                                start=(fc == fcs[0]), stop=(fc == fcs[1]),
                            )
                    ysb = wk.tile([128, T], MMDT, tag="ysb")
                    if ec8 % 2 == 0:
                        nc.scalar.copy(ysb[:], yps[:])
                    else:
                        nc.vector.tensor_copy(ysb[:], yps[:])
                    nc.sync.dma_start(out_t[ec8 * 128:(ec8 + 1) * 128, :], ysb[:])
                return go

            # ---- prologue: q/k fc0 inline ----
            for cl in proj_closures('q', 0) + proj_closures('k', 0):
                cl()

            # ---- pipelined pair cycles ----
            for hc in range(4):
                if hc == 0:
                    new = [v_closure(sc) for sc in range(8)]
                    new += proj_closures('q', 1) + proj_closures('k', 1)
                else:
                    pvA = pv_closures(2 * (hc - 1))
                    pvB = pv_closures(2 * (hc - 1) + 1)
                    new = []
                    for i in range(9):
                        new.append(pvA[i])
                        new.append(pvB[i])
                    if hc < 3:
                        new += proj_closures('q', hc + 1) + proj_closures('k', hc + 1)
                fillers.extend(new)
                for sc in range(8):
                    qk_step(hc, sc)
                    rem = len(fillers)
                    drain(-(-rem // (8 - sc)))
                drain(len(fillers))

            # ---- epilogue: PV pair 3 + full output projection ----
            pv6, pv7 = pv_closures(6), pv_closures(7)
            y01 = [y_closure(0, ec8, yTa) for ec8 in range(8)]
            for i in range(4):
                pv6[i]()
                y01[i]()
            for i in range(4, 9):
                pv6[i]()
            for i in range(4):
                pv7[i]()
                y01[4 + i]()
            for i in range(4, 9):
                pv7[i]()
            for ec8 in range(8):
                y_closure(1, ec8, yT)()

    nc.compile()
    return nc


_NC_CACHE = []


def kernel(query, key_, value, edge_bias, attn_mask, key_padding_mask,
           Wq, bq, Wk, bk, Wv, bv, Wo, bo):
    if not _NC_CACHE:
        _NC_CACHE.append(_build_program())
    nc = _NC_CACHE[0]

    scale = np.float32(D ** -0.5)
    q32, k32, v32 = (np.asarray(a, np.float32) for a in (query, key_, value))
    WqT = (np.asarray(Wq, np.float32).T * scale).astype(NP_MMDT)
    WkT = np.asarray(Wk, np.float32).T.astype(NP_MMDT)
    WvT = np.asarray(Wv, np.float32).T.astype(NP_MMDT)
    WoT = np.asarray(Wo, np.float32).T
    bq_s = (np.asarray(bq, np.float32) * scale)
    kpm_add = np.where(np.asarray(key_padding_mask), np.float32(-1e30),
                       np.float32(0.0))  # [B, S]
    mask32 = np.asarray(attn_mask, np.float32)

    in_maps = []
    for c in range(N_CORES):
        b, g = divmod(c, 2)
        cols = slice(g * F, (g + 1) * F)
        bias_sb = (mask32 + np.asarray(edge_bias[b], np.float32)
                   + kpm_add[b][None, :])  # [T, S]
        in_maps.append({
            "xqT": np.ascontiguousarray(q32[b].T).astype(NP_MMDT),
            "xkT": np.ascontiguousarray(k32[b].T).astype(NP_MMDT),
            "xvT": np.ascontiguousarray(v32[b].T).astype(NP_MMDT),
            "wqT": np.ascontiguousarray(WqT[:, cols]),
            "wkT": np.ascontiguousarray(WkT[:, cols]),
            "wvT": np.ascontiguousarray(WvT[:, cols]),
            "woT": np.ascontiguousarray(WoT[cols, :]).astype(NP_MMDT),
            "bq": np.ascontiguousarray(bq_s[cols].reshape(4, 128).T),
            "bk": np.ascontiguousarray(np.asarray(bk, np.float32)[cols]
                                       .reshape(4, 128).T),
            "bv": np.asarray(bv, np.float32)[cols].reshape(1, F),
            "expbT": np.exp(bias_sb.T).astype(NP_MMDT),
        })

    res = run_bass_kernel_spmd(nc, in_maps, list(range(N_CORES)))

    out = np.empty((B, T, E), np.float32)
    bo32 = np.asarray(bo, np.float32)
    for b in range(B):
        r0, r1 = res.results[2 * b], res.results[2 * b + 1]
        acc = (r0["yT"].astype(np.float32) + r0["yTa"].astype(np.float32)
               + r1["yT"].astype(np.float32) + r1["yTa"].astype(np.float32))
        out[b] = acc.T + bo32[None, :]
    return out


# revision 10
# speedup vs baseline: 1.0409x; 1.0186x over previous
"""Graphormer multi-head attention on 8 Trainium2 cores.

Sharding: 2 cores per batch element (B=4), each core handling 8 of 16 heads
(tensor-parallel within the batch). Per core:
  - QKV projections for its 512 local feature columns (transposed layouts)
  - scoresT[s,t] = K_h Q_h^T per head (K=64 contraction on PE)
  - p = exp(scoresT) * expbT  (expbT = exp(attn_mask + edge_bias).T from host)
  - PV with a ones-column appended to V -> row 64 of PSUM = softmax denom
  - normalize via partition-broadcast + reciprocal, out-project 512 local
    features into three bf16 partials (fc0+fc1, fc2, fc3), summed on host.
PV(h-1) matmuls interleave into QK(h)'s loop to keep the PE dense. The
output projection is split so only the fc3 quarter remains after the last
head (fc0+fc1 during heads 5-6, fc2 during head 7). PV accumulators are
th-granular [65, 512] so the shared PSUM ring gets 3 buffers (deeper
QK-psum pipelining). Inputs DMA in per-chunk pieces so the first matmul
starts early; a dummy exp preloads the ACT table during the DMA wait.
"""
import sys

sys.path.insert(0, '/opt/trn_rl_repo')

import ml_dtypes
import numpy as np

import concourse.bass as bass
import concourse.mybir as mybir
import concourse.tile as tile
from concourse import bacc
from concourse.bass_utils import run_bass_kernel_spmd

DT = mybir.dt

B, T, S, E, H = 4, 1024, 1024, 1024, 16
D = E // H          # 64
HL = 8              # heads per core
F = HL * D          # 512 local features
N_CORES = 8

MMDT = DT.bfloat16
NP_MMDT = ml_dtypes.bfloat16


def _build_program():
    nc = bacc.Bacc()

    xqT = nc.dram_tensor("xqT", [E, T], MMDT, kind="ExternalInput")
    xkT = nc.dram_tensor("xkT", [E, S], MMDT, kind="ExternalInput")
    xvT = nc.dram_tensor("xvT", [E, S], MMDT, kind="ExternalInput")
    wqT = nc.dram_tensor("wqT", [E, F], MMDT, kind="ExternalInput")
    wkT = nc.dram_tensor("wkT", [E, F], MMDT, kind="ExternalInput")
    wvT = nc.dram_tensor("wvT", [E, F], MMDT, kind="ExternalInput")
    woT = nc.dram_tensor("woT", [F, E], MMDT, kind="ExternalInput")
    bq = nc.dram_tensor("bq", [128, 4], DT.float32, kind="ExternalInput")
    bk = nc.dram_tensor("bk", [128, 4], DT.float32, kind="ExternalInput")
    bv = nc.dram_tensor("bv", [1, F], DT.float32, kind="ExternalInput")
    expbT = nc.dram_tensor("expbT", [S, T], MMDT, kind="ExternalInput")
    yT = nc.dram_tensor("yT", [E, T], MMDT, kind="ExternalOutput")
    yTa = nc.dram_tensor("yTa", [E, T], MMDT, kind="ExternalOutput")
    yTb = nc.dram_tensor("yTb", [E, T], MMDT, kind="ExternalOutput")

    with tile.TileContext(nc) as tc:
        with tc.tile_pool(name="persist", bufs=1) as pp, \
             tc.tile_pool(name="xin", bufs=2) as xp, \
             tc.tile_pool(name="work", bufs=3) as wk, \
             tc.tile_pool(name="pT", bufs=2) as pTp, \
             tc.tile_pool(name="ps", bufs=3, space="PSUM") as ps, \
             tc.tile_pool(name="pspv", bufs=2, space="PSUM") as pspv:

            # ---- ACT exp-table preload during the DMA wait ----
            warm_in = pp.tile([1, 8], DT.float32, tag="warm_in")
            warm_out = pp.tile([1, 8], DT.float32, tag="warm_out")
            nc.vector.memset(warm_in[:], 0.0)
            nc.scalar.activation(warm_out[:], warm_in[:],
                                 mybir.ActivationFunctionType.Exp)

            # ---- q projection inputs first (critical path) ----
            wq_sb = pp.tile([128, 8, F], MMDT, tag="wq")
            wqT_v = wqT[:].rearrange("(c p) f -> p c f", p=128)
            bq_sb = pp.tile([128, 4], DT.float32, tag="bq")
            qT_sb = pp.tile([128, 4, T], MMDT, tag="qT")
            kT_sb = pp.tile([128, 4, S], MMDT, tag="kT")
            xq_sb = xp.tile([128, 8, T], MMDT, tag="x")
            xqT_v = xqT[:].rearrange("(c p) t -> p c t", p=128)
            for ec in range(8):
                nc.sync.dma_start(wq_sb[:, ec, :], wqT_v[:, ec, :])
                nc.sync.dma_start(xq_sb[:, ec, :], xqT_v[:, ec, :])
            nc.sync.dma_start(bq_sb[:], bq[:])

            wk_sb = pp.tile([128, 8, F], MMDT, tag="wk")
            wkT_v = wkT[:].rearrange("(c p) f -> p c f", p=128)
            bk_sb = pp.tile([128, 4], DT.float32, tag="bk")
            xk_sb = xp.tile([128, 8, S], MMDT, tag="x")
            xkT_v = xkT[:].rearrange("(c p) t -> p c t", p=128)
            for ec in range(8):
                nc.sync.dma_start(wk_sb[:, ec, :], wkT_v[:, ec, :])
                nc.sync.dma_start(xk_sb[:, ec, :], xkT_v[:, ec, :])
            nc.sync.dma_start(bk_sb[:], bk[:])

            for x_sb, w_sb, b_sb, dst in (
                (xq_sb, wq_sb, bq_sb, qT_sb),
                (xk_sb, wk_sb, bk_sb, kT_sb),
            ):
                for fc in range(4):
                    acc = ps.tile([128, T], DT.float32, tag="mm")
                    for th in range(2):
                        for ec in range(8):
                            nc.tensor.matmul(
                                acc[:, th * 512:(th + 1) * 512],
                                w_sb[:, ec, fc * 128:(fc + 1) * 128],
                                x_sb[:, ec, th * 512:(th + 1) * 512],
                                start=(ec == 0), stop=(ec == 7),
                            )
                    nc.scalar.add(dst[:, fc, :], acc[:], b_sb[:, fc:fc + 1])

            # ---- v projection into ones-augmented layout [s, h*65+d] ----
            wv_sb = pp.tile([128, 8, F], MMDT, tag="wv")
            wvT_v = wvT[:].rearrange("(c p) f -> p c f", p=128)
            xv_sb = xp.tile([128, 8, S], MMDT, tag="x")
            xvT_v = xvT[:].rearrange("(c p) s -> p c s", p=128)
            for ec in range(8):
                nc.sync.dma_start(wv_sb[:, ec, :], wvT_v[:, ec, :])
                nc.sync.dma_start(xv_sb[:, ec, :], xvT_v[:, ec, :])
            bv_row = pp.tile([1, F], DT.float32, tag="bvrow")
            nc.sync.dma_start(bv_row[:], bv[:])
            bv_bc = pp.tile([128, F], DT.float32, tag="bvbc")
            nc.gpsimd.partition_broadcast(bv_bc[:], bv_row[:])
            v_sb = pp.tile([128, 8, HL * 65], MMDT, tag="v")
            for sc in range(8):
                acc = ps.tile([128, T], DT.float32, tag="mm")
                for ec in range(8):
                    nc.tensor.matmul(
                        acc[:, 0:F],
                        xv_sb[:, ec, sc * 128:(sc + 1) * 128],
                        wv_sb[:, ec, :],
                        start=(ec == 0), stop=(ec == 7),
                    )
                vv = v_sb[:, sc, :].rearrange("p (h c) -> p h c", c=65)
                nc.vector.tensor_add(
                    vv[:, :, 0:64],
                    acc[:, 0:F].rearrange("p (h d) -> p h d", d=64),
                    bv_bc[:].rearrange("p (h d) -> p h d", d=64),
                )
                nc.vector.memset(vv[:, :, 64:65], 1.0)

            # ---- exp(biasT) from host ----
            expb_sb = pp.tile([128, 8, T], MMDT, tag="expb")
            expbT_v = expbT[:].rearrange("(c p) t -> p c t", p=128)
            for ec in range(8):
                nc.sync.dma_start(expb_sb[:, ec, :], expbT_v[:, ec, :])

            wo_sb = pp.tile([128, 4, E], MMDT, tag="wo")
            nc.sync.dma_start(wo_sb[:], woT[:].rearrange("(c p) e -> p c e", p=128))

            # ---- attention: QK(h) interleaved with PV(h-1) MMs ----
            oT_sb = pp.tile([128, 4, T], MMDT, tag="oT")
            state = {}

            def emit_qk_chunk(h, sc):
                hc, po = h // 2, 64 * (h % 2)
                if sc == 0:
                    pT_new = pTp.tile([128, 8, T], MMDT, tag="pT")
                    state[h] = pT_new
                pT = state[h]
                sps = ps.tile([128, T], DT.float32, tag="mm")
                for th in range(2):
                    nc.tensor.matmul(
                        sps[:, th * 512:(th + 1) * 512],
                        kT_sb[po:po + 64, hc, sc * 128:(sc + 1) * 128],
                        qT_sb[po:po + 64, hc, th * 512:(th + 1) * 512],
                        start=True, stop=True,
                    )
                et = wk.tile([128, T], MMDT, tag="exps")
                nc.scalar.activation(et[:], sps[:],
                                     mybir.ActivationFunctionType.Exp)
                nc.vector.tensor_mul(pT[:, sc, :], et[:], expb_sb[:, sc, :])

            def emit_pv_chunk(h, i):
                # i in 0..7 -> PV matmuls 2i, 2i+1 of head h (16 total)
                pT = state[h]
                if i == 0:
                    a0 = pspv.tile([65, 512], DT.float32, tag="pv", name="pva")
                    a1 = pspv.tile([65, 512], DT.float32, tag="pv", name="pvb")
                    state[(h, "acc")] = (a0, a1)
                    osb = wk.tile([64, T], DT.float32, tag="osb", bufs=2)
                    lrow = wk.tile([1, T], DT.float32, tag="lrow", bufs=2)
                    state[(h, "o")] = (osb, lrow)
                a0, a1 = state[(h, "acc")]
                osb, lrow = state[(h, "o")]
                for j in (2 * i, 2 * i + 1):
                    th, sc = divmod(j, 8)
                    acc = a0 if th == 0 else a1
                    nc.tensor.matmul(
                        acc[:],
                        v_sb[:, sc, h * 65:(h + 1) * 65],
                        pT[:, sc, th * 512:(th + 1) * 512],
                        start=(sc == 0), stop=(sc == 7),
                    )
                    if sc == 7:
                        # evacuate this th-half so the bank frees early
                        nc.vector.tensor_copy(
                            osb[:, th * 512:(th + 1) * 512], acc[0:64, :])
                        nc.vector.tensor_copy(
                            lrow[:, th * 512:(th + 1) * 512], acc[64:65, :])

            def emit_norm(h):
                hc, po = h // 2, 64 * (h % 2)
                del state[(h, "acc")]
                del state[h]
                osb, lrow = state.pop((h, "o"))
                rlb = wk.tile([64, T], DT.float32, tag="rlb", bufs=2)
                nc.gpsimd.partition_broadcast(rlb[:], lrow[:])
                rli = wk.tile([64, T], DT.float32, tag="rli", bufs=2)
                nc.vector.reciprocal_approx_fast(out=rli[:], in_=rlb[:])
                nc.vector.tensor_mul(oT_sb[po:po + 64, hc, :], osb[:], rli[:])

            # output projection quarters/halves, pipelined into the loop:
            # yTa = fc0+fc1 (heads 5-6), yTb = fc2 (head 7), yT = fc3 (tail)
            def emit_y01_chunk(slot):
                ec8, th = divmod(slot, 2)
                if th == 0:
                    yps_new = ps.tile([128, T], DT.float32, tag="mm")
                    state[("y", ec8)] = yps_new
                yps = state[("y", ec8)]
                for fc in range(2):
                    nc.tensor.matmul(
                        yps[:, th * 512:(th + 1) * 512],
                        wo_sb[:, fc, ec8 * 128:(ec8 + 1) * 128],
                        oT_sb[:, fc, th * 512:(th + 1) * 512],
                        start=(fc == 0), stop=(fc == 1),
                    )
                if th == 1:
                    ya = wk.tile([128, T], MMDT, tag="yout")
                    nc.vector.tensor_copy(ya[:], state.pop(("y", ec8))[:])
                    nc.sync.dma_start(yTa[ec8 * 128:(ec8 + 1) * 128, :], ya[:])

            def emit_yq_chunk(fc, ec8, out_t, copy_eng):
                yps = ps.tile([128, T], DT.float32, tag="mm")
                for th in range(2):
                    nc.tensor.matmul(
                        yps[:, th * 512:(th + 1) * 512],
                        wo_sb[:, fc, ec8 * 128:(ec8 + 1) * 128],
                        oT_sb[:, fc, th * 512:(th + 1) * 512],
                        start=True, stop=True,
                    )
                ya = wk.tile([128, T], MMDT, tag="yout")
                if copy_eng == 'scalar':
                    nc.scalar.copy(ya[:], yps[:])
                else:
                    nc.vector.tensor_copy(ya[:], yps[:])
                nc.sync.dma_start(out_t[ec8 * 128:(ec8 + 1) * 128, :], ya[:])

            for sc in range(8):
                emit_qk_chunk(0, sc)
            for h in range(1, HL):
                for sc in range(8):
                    emit_qk_chunk(h, sc)
                    emit_pv_chunk(h - 1, sc)
                    if h in (5, 6):
                        emit_y01_chunk((h - 5) * 8 + sc)
                    if h == 7:
                        emit_yq_chunk(2, sc, yTb, 'vector')
                emit_norm(h - 1)
            for i in range(8):
                emit_pv_chunk(HL - 1, i)
            emit_norm(HL - 1)

            # ---- tail: fc3 quarter only ----
            for ec8 in range(8):
                emit_yq_chunk(3, ec8, yT, 'scalar')

    nc.compile()
    return nc


_NC_CACHE = []


def kernel(query, key_, value, edge_bias, attn_mask, key_padding_mask,
           Wq, bq, Wk, bk, Wv, bv, Wo, bo):
    if not _NC_CACHE:
        _NC_CACHE.append(_build_program())
    nc = _NC_CACHE[0]

    scale = np.float32(D ** -0.5)
    q32, k32, v32 = (np.asarray(a, np.float32) for a in (query, key_, value))
    WqT = (np.asarray(Wq, np.float32).T * scale).astype(NP_MMDT)
    WkT = np.asarray(Wk, np.float32).T.astype(NP_MMDT)
    WvT = np.asarray(Wv, np.float32).T.astype(NP_MMDT)
    WoT = np.asarray(Wo, np.float32).T
    bq_s = (np.asarray(bq, np.float32) * scale)
    kpm_add = np.where(np.asarray(key_padding_mask), np.float32(-1e30),
                       np.float32(0.0))  # [B, S]
    mask32 = np.asarray(attn_mask, np.float32)

    in_maps = []
    for c in range(N_CORES):
        b, g = divmod(c, 2)
        cols = slice(g * F, (g + 1) * F)
        bias_sb = (mask32 + np.asarray(edge_bias[b], np.float32)
                   + kpm_add[b][None, :])  # [T, S]
        in_maps.append({
            "xqT": np.ascontiguousarray(q32[b].T).astype(NP_MMDT),
            "xkT": np.ascontiguousarray(k32[b].T).astype(NP_MMDT),
            "xvT": np.ascontiguousarray(v32[b].T).astype(NP_MMDT),
            "wqT": np.ascontiguousarray(WqT[:, cols]),
            "wkT": np.ascontiguousarray(WkT[:, cols]),
            "wvT": np.ascontiguousarray(WvT[:, cols]),
            "woT": np.ascontiguousarray(WoT[cols, :]).astype(NP_MMDT),
            "bq": np.ascontiguousarray(bq_s[cols].reshape(4, 128).T),
            "bk": np.ascontiguousarray(np.asarray(bk, np.float32)[cols]
                                       .reshape(4, 128).T),
            "bv": np.asarray(bv, np.float32)[cols].reshape(1, F),
            "expbT": np.exp(bias_sb.T).astype(NP_MMDT),
        })

    res = run_bass_kernel_spmd(nc, in_maps, list(range(N_CORES)))

    out = np.empty((B, T, E), np.float32)
    bo32 = np.asarray(bo, np.float32)
    for b in range(B):
        r0, r1 = res.results[2 * b], res.results[2 * b + 1]
        acc = (r0["yT"].astype(np.float32) + r0["yTa"].astype(np.float32)
               + r0["yTb"].astype(np.float32) + r1["yT"].astype(np.float32)
               + r1["yTa"].astype(np.float32) + r1["yTb"].astype(np.float32))
        out[b] = acc.T + bo32[None, :]
    return out
